# revision 52
# baseline (speedup 1.0000x reference)
"""Trainium2 Bass kernel for nn_ContinuousGenHyperConnections.

Sharding: data-parallel over the batch dim B=8192 across 8 NeuronCores
(1024 rows each). All weights replicated; no collectives.

Per-core dataflow (B_loc=1024 -> 8 b-tiles of 128 rows; proj in blocks of
TPB=4 tiles; block 0's chunk loads are interleaved with its P3 chunks):
  P1 per tile : DMA x fp32 chunks (scalar queue); cast->bf16 into resident
                x_bf (DVE/ACT); sum-of-squares via ACT Square accum + DVE
                STT accum; s = 1/sqrt(mean+eps).
  P3 per block: x transposed ON-CHIP: PE transposes 2 chunks x 4 tiles
                into a bf16 psum bank, one [128,2,512] copy out, then
                bf16 matmuls accumulate proj.T [42,512]; PE-transpose
                + scale by s -> proj_all (no DRAM round-trip).
  P4 per block: per-row 4x4 generator math batched over 4 tiles (batched
                4x4 matmuls on DVE, order-3 Taylor expm + 4 squarings),
                rw/ww, c = E^T rw; E shuffled to (j,b') partition layout
                (E_grp) via tiny gpsimd DMAs.
  P5 per tile : branch = sum_j c_j x_j (ACT mults + DVE adds),
                16 PE-transposes -> brT fp8 (batched psum copies).
  P6 per tile : y = branch @ W_mod.T via fp8 DoubleRow matmuls
                (2 k-chunks per instr at 0.5 cyc/row) -> y_nb [128,2048].
  P7 per tile : out = E x + diag(ww) y on the PE: x is regrouped per
                32-row group g to partitions (j,b') (16 gpsimd SBUF-SBUF
                DMAs/tile, issued early); stationary Eblk[(j,b'),(n,b'')]
                = E_nj[b]*delta builds via one jpat TT per group; a K=128
                matmul mixes all 4 streams for 32 rows at full PE width,
                and a K=32 matmul at partition offset 32g adds ww_n y into
                the same psum; psum -> fp32 out copies alternate DVE/ACT;
                out DMA (sync queue) uses a transposed 3D DRAM AP.
"""

import os
import sys

sys.path.insert(0, "/opt/trn_rl_repo")

import numpy as np
import ml_dtypes

BF16 = ml_dtypes.bfloat16

DT_MIN, DT_MAX = 1e-3, 1.0
EPS = 1e-6
NS = 4  # streams
EMB = 2048
IN_DIM = 8192
N_CORES = 8
NPROJ = 42  # 16 conv + 16 diss + 1 dtc + 1 dtd + 4 read + 4 write


def _build(B_loc, scal, num_devices=N_CORES):
    import concourse.bacc as bacc
    import concourse.mybir as mybir
    import concourse.tile as tile
    from concourse.masks import make_identity
    from contextlib import ExitStack

    dt = mybir.dt
    Alu = mybir.AluOpType
    Act = mybir.ActivationFunctionType
    Axis = mybir.AxisListType
    DR = mybir.MatmulPerfMode.DoubleRow

    NT = B_loc // 128
    TPB = min(4, NT)          # tiles per proj block
    NBLK = NT // TPB
    NCH = IN_DIM // 128       # 64 contraction chunks
    NB = TPB * 128            # rows per proj block

    # expm 2^-4 prescale folded into dt: dt_eff = (DT_MIN + range*sig)/16
    R_SIG = (DT_MAX - DT_MIN) / 16.0
    C_SIG = DT_MIN / 16.0

    nc = bacc.Bacc("TRN2", target_bir_lowering=False, debug=False,
                   num_devices=num_devices)

    x_ext = nc.declare_dram_parameter("x", [B_loc, IN_DIM], dt.float32,
                                      isOutput=False)
    wcatT_ext = nc.declare_dram_parameter("wcatT", [128, NCH, NPROJ],
                                          dt.bfloat16, isOutput=False)
    wmodT_ext = nc.declare_dram_parameter("wmodT", [128, 16, EMB],
                                          dt.float8e4, isOutput=False)
    cpack_ext = nc.declare_dram_parameter("cpack", [58], dt.float32,
                                          isOutput=False)
    jpat_ext = nc.declare_dram_parameter("jpat", [128, 128], dt.bfloat16,
                                         isOutput=False)
    out_ext = nc.declare_dram_parameter("out", [B_loc, NS, EMB], dt.float32,
                                        isOutput=True)

    with tile.TileContext(nc) as tc, ExitStack() as ctx:
        const_pool = ctx.enter_context(tc.tile_pool(name="const", bufs=1))
        p1_pool = ctx.enter_context(tc.tile_pool(name="p1", bufs=2))
        xbb_pool = ctx.enter_context(tc.tile_pool(name="xbb", bufs=4))
        xt_pool = ctx.enter_context(tc.tile_pool(name="xt", bufs=2))
        small_pool = ctx.enter_context(tc.tile_pool(name="small", bufs=2))
        sm1_pool = ctx.enter_context(tc.tile_pool(name="sm1", bufs=1))
        str_pool = ctx.enter_context(tc.tile_pool(name="stream", bufs=2))
        brt_pool = ctx.enter_context(tc.tile_pool(name="brt", bufs=2))
        out_pool = ctx.enter_context(tc.tile_pool(name="outp", bufs=2))
        xg_pool = ctx.enter_context(tc.tile_pool(name="xg", bufs=6))
        blk_pool = ctx.enter_context(tc.tile_pool(name="blk", bufs=5))
        ps_proj = ctx.enter_context(
            tc.tile_pool(name="ps_proj", bufs=1, space="PSUM"))
        ps_trp = ctx.enter_context(
            tc.tile_pool(name="ps_trp", bufs=1, space="PSUM"))
        ps_br = ctx.enter_context(
            tc.tile_pool(name="ps_br", bufs=2, space="PSUM"))
        ps_y = ctx.enter_context(
            tc.tile_pool(name="ps_y", bufs=4, space="PSUM"))
        ps_out = ps_y

        # ---- constants ----
        wcatT = const_pool.tile([128, NCH, NPROJ], dt.bfloat16)
        nc.sync.dma_start(wcatT[:], wcatT_ext[:])
        wmodT = const_pool.tile([128, 16, EMB], dt.float8e4)
        nc.scalar.dma_start(wmodT[:], wmodT_ext[:])
        cpk = const_pool.tile([128, 58], dt.float32)
        nc.sync.dma_start(cpk[:], cpack_ext[:].partition_broadcast(128))
        jpat = const_pool.tile([128, 128], dt.bfloat16)
        nc.sync.dma_start(jpat[:], jpat_ext[:])
        ident_bf = const_pool.tile([128, 128], dt.bfloat16)
        make_identity(nc, ident_bf[:])
        ident_f32 = const_pool.tile([128, 128], dt.float32)
        make_identity(nc, ident_f32[:])

        skew_c = cpk[:, 0:16]     # (conservA+bconv) - transpose, flattened
        diss_c = cpk[:, 16:32]    # dissA + bdiss, flattened
        eye16 = cpk[:, 32:48]     # flattened I4
        readin_c = cpk[:, 48:52]
        writeout_c = cpk[:, 52:56]

        s_all = sm1_pool.tile([128, NT], dt.float32)
        proj_all = sm1_pool.tile([128, NT, NPROJ], dt.float32)
        E_all = sm1_pool.tile([128, NT, 16], dt.float32)
        E_grp = sm1_pool.tile([128, 4, NT, NS], dt.float32)
        c_all = sm1_pool.tile([128, NT, NS], dt.float32)
        ww_all = sm1_pool.tile([128, NT, NS], dt.float32)
        ss_all = sm1_pool.tile([128, NT, 4], dt.float32)

        def bcast(ap2d, shape):
            return ap2d.unsqueeze(1).broadcast_to(shape)

        x_bfs = {}

        x_grps = {}
        brTs = {}

        p1_state = {}

        def p1_chunk(t, q):
            """load + cast + sum-of-squares for chunk q of tile t."""
            if q == 0:
                x_bf = xbb_pool.tile([128, IN_DIM], dt.bfloat16,
                                     tag="x_bf", name=f"x_bf{t}")
                x_bfs[t] = x_bf
            x_bf = x_bfs[t]
            ss = ss_all[:, t, :]
            xf = p1_pool.tile([128, EMB], dt.float32, tag="xf")
            eng = nc.sync if (t < TPB and q % 2 == 0) else nc.scalar
            eng.dma_start(
                xf[:], x_ext[t * 128:(t + 1) * 128,
                             q * EMB:(q + 1) * EMB])
            if q % 2 == 0:
                nc.vector.tensor_copy(x_bf[:, q * EMB:(q + 1) * EMB],
                                      xf[:])
                sqj = str_pool.tile([128, EMB], dt.bfloat16, tag="tmp")
                nc.scalar.activation(sqj[:], xf[:], Act.Square,
                                     accum_out=ss[:, q:q + 1])
            else:
                nc.scalar.activation(x_bf[:, q * EMB:(q + 1) * EMB],
                                     xf[:], Act.Copy)
                sqj = str_pool.tile([128, EMB], dt.bfloat16, tag="tmp")
                nc.vector.scalar_tensor_tensor(
                    out=sqj[:], in0=x_bf[:, q * EMB:(q + 1) * EMB],
                    scalar=1.0, in1=x_bf[:, q * EMB:(q + 1) * EMB],
                    op0=Alu.bypass, op1=Alu.mult,
                    accum_out=ss[:, q:q + 1])

        def p1_finish(t):
            ssum = small_pool.tile([128, 1], dt.float32, tag="s01")
            nc.vector.tensor_reduce(ssum[:], ss_all[:, t, :], Axis.X,
                                    Alu.add)
            nc.vector.tensor_scalar(
                out=ssum[:], in0=ssum[:], scalar1=1.0 / IN_DIM,
                scalar2=EPS, op0=Alu.mult, op1=Alu.add)
            sqr = small_pool.tile([128, 1], dt.float32, tag="sqr")
            nc.scalar.activation(sqr[:], ssum[:], Act.Sqrt)
            nc.vector.reciprocal(s_all[:, t:t + 1], sqr[:])

        def p1_tile(t):
            for q in range(4):
                p1_chunk(t, q)
            p1_finish(t)

        p3_state = {}

        def p3_start(g):
            p3_state[g] = ps_proj.tile([NPROJ, NB], dt.float32,
                                       tag="proj_ps", name=f"proj_ps{g}")

        def p3_part(g, c0, c1):
            """proj.T chunks [c0,c1) via on-chip PE transposes + matmul."""
            proj_ps = p3_state[g]
            for c2 in range(c0 // 2, c1 // 2):
                # transpose 2 chunks x 4 tiles into one bf16 psum bank
                tp = ps_br.tile([128, 8, 128], dt.bfloat16, tag="br_ps",
                                name=f"xtp{g}_{c2}")
                for cc in (2 * c2, 2 * c2 + 1):
                    for i in range(TPB):
                        nc.tensor.transpose(
                            tp[:, (cc % 2) * TPB + i, :],
                            x_bfs[g * TPB + i][:, cc * 128:(cc + 1) * 128],
                            ident_bf[:])
                xt = xt_pool.tile([128, 2, NB], dt.bfloat16, tag="xt")
                if c2 % 2 == 0:
                    nc.vector.tensor_copy(xt[:], tp[:])
                else:
                    nc.scalar.activation(xt[:], tp[:], Act.Copy)
                for cc in (2 * c2, 2 * c2 + 1):
                    nc.tensor.matmul(proj_ps[:], wcatT[:, cc, :],
                                     xt[:, cc % 2, :],
                                     start=(cc == 0), stop=(cc == NCH - 1))

        def p3_finish(g):
            proj_ps = p3_state.pop(g)
            projT = sm1_pool.tile([NPROJ, NB], dt.float32, tag="projT")
            nc.vector.tensor_copy(projT[:], proj_ps[:])
            for i in range(TPB):
                t = g * TPB + i
                tr_ps = ps_trp.tile([128, NPROJ], dt.float32, tag="tr_ps")
                nc.tensor.transpose(
                    tr_ps[:], projT[:, i * 128:(i + 1) * 128],
                    ident_f32[:NPROJ, :NPROJ])
                nc.vector.tensor_scalar(
                    out=proj_all[:, t, :], in0=tr_ps[:],
                    scalar1=s_all[:, t:t + 1], scalar2=None, op0=Alu.mult)

        def p3_proj(g):
            p3_start(g)
            p3_part(g, 0, NCH)
            p3_finish(g)

        def p4_smalls(g):
            """per-row generator math for block g, batched over TPB tiles."""
            pb = proj_all[:, g * TPB:(g + 1) * TPB, :]   # [128,TPB,42]

            def mm2(dst, lhs, rhs, rhs_pat):
                # batched per-row 4x4 matmul: loop j (broadcast dim) only
                pr = small_pool.tile([128, TPB, 64], dt.float32, tag="prod")
                pv5 = pr[:].rearrange("p t (i j k) -> p t i j k",
                                      j=NS, k=NS)
                lv = lhs[:].rearrange("p t (i k) -> p t i k", k=NS)
                rv = rhs[:].rearrange(rhs_pat, j=NS)
                for j in range(NS):
                    nc.vector.tensor_tensor(
                        pv5[:, :, :, j, :], lv,
                        rv[:, :, j, :].unsqueeze(2)
                        .broadcast_to([128, TPB, NS, NS]),
                        Alu.mult)
                nc.vector.tensor_reduce(
                    dst[:], pr[:].rearrange("p t (ij k) -> p t ij k", k=NS),
                    Axis.X, Alu.add)

            smw = small_pool.tile([128, TPB, 16], dt.float32, tag="smw")
            nc.vector.tensor_tensor(
                smw[:].rearrange("p t (i j) -> p t i j", j=NS),
                pb[:, :, 0:16].rearrange("p t (i j) -> p t i j", j=NS),
                pb[:, :, 0:16].rearrange("p t (j i) -> p t i j", i=NS),
                Alu.subtract)
            nc.vector.tensor_tensor(smw[:], smw[:],
                                    bcast(skew_c, [128, TPB, 16]), Alu.add)
            Rm = small_pool.tile([128, TPB, 16], dt.float32, tag="Rm")
            nc.vector.tensor_tensor(Rm[:], pb[:, :, 16:32],
                                    bcast(diss_c, [128, TPB, 16]), Alu.add)
            dtc = small_pool.tile([128, TPB, 1], dt.float32, tag="dtc")
            dtd = small_pool.tile([128, TPB, 1], dt.float32, tag="dtd")
            nc.scalar.activation(dtc[:], pb[:, :, 32:33], Act.Sigmoid,
                                 bias=cpk[:, 56:57])
            nc.scalar.activation(dtd[:], pb[:, :, 33:34], Act.Sigmoid,
                                 bias=cpk[:, 57:58])
            nc.vector.tensor_scalar(out=dtc[:], in0=dtc[:], scalar1=R_SIG,
                                    scalar2=C_SIG, op0=Alu.mult, op1=Alu.add)
            nc.vector.tensor_scalar(out=dtd[:], in0=dtd[:], scalar1=R_SIG,
                                    scalar2=C_SIG, op0=Alu.mult, op1=Alu.add)

            # K = R @ R^T
            Km = small_pool.tile([128, TPB, 16], dt.float32, tag="Km")
            mm2(Km, Rm, Rm, "p t (j k) -> p t j k")
            # A = dtc*skew - dtd*K  (batched: broadcast dt over the 16 cols)
            Am = small_pool.tile([128, TPB, 16], dt.float32, tag="Am")
            tmpA = small_pool.tile([128, TPB, 16], dt.float32, tag="tmpA")
            nc.vector.tensor_tensor(
                Am[:], smw[:], dtc[:].broadcast_to([128, TPB, 16]), Alu.mult)
            nc.vector.tensor_tensor(
                tmpA[:], Km[:], dtd[:].broadcast_to([128, TPB, 16]),
                Alu.mult)
            nc.vector.tensor_tensor(Am[:], Am[:], tmpA[:], Alu.subtract)
            # expm: order-4 Taylor (||A/16|| <= ~1.1 -> err < 2e-4) + 4 sq
            A2 = small_pool.tile([128, TPB, 16], dt.float32, tag="A2")
            A3 = small_pool.tile([128, TPB, 16], dt.float32, tag="A3")
            mm2(A2, Am, Am, "p t (k j) -> p t j k")
            mm2(A3, A2, Am, "p t (k j) -> p t j k")
            Em = small_pool.tile([128, TPB, 16], dt.float32, tag="Em")
            nc.vector.tensor_tensor(Em[:], Am[:],
                                    bcast(eye16, [128, TPB, 16]), Alu.add)
            nc.vector.scalar_tensor_tensor(
                out=Em[:], in0=A2[:], scalar=0.5, in1=Em[:],
                op0=Alu.mult, op1=Alu.add)
            nc.vector.scalar_tensor_tensor(
                out=Em[:], in0=A3[:], scalar=1.0 / 6.0, in1=Em[:],
                op0=Alu.mult, op1=Alu.add)
            E2 = small_pool.tile([128, TPB, 16], dt.float32, tag="E2")
            cur, nxt = Em, E2
            for q in range(4):
                if q == 3:
                    mm2(E_all[:, g * TPB:(g + 1) * TPB, :], cur, cur,
                        "p t (k j) -> p t j k")
                else:
                    mm2(nxt, cur, cur, "p t (k j) -> p t j k")
                    cur, nxt = nxt, cur
            Ev = E_all[:, g * TPB:(g + 1) * TPB, :]
            # rw / ww / c
            rw = small_pool.tile([128, TPB, NS], dt.float32, tag="rw")
            nc.vector.tensor_scalar(out=rw[:], in0=pb[:, :, 34:38],
                                    scalar1=scal["alpha_r"], scalar2=None,
                                    op0=Alu.mult)
            nc.vector.tensor_tensor(rw[:], rw[:],
                                    bcast(readin_c, [128, TPB, NS]), Alu.add)
            nc.scalar.activation(rw[:], rw[:], Act.Sigmoid)
            wws = ww_all[:, g * TPB:(g + 1) * TPB, :]
            nc.vector.tensor_scalar(out=wws, in0=pb[:, :, 38:42],
                                    scalar1=scal["alpha_w"], scalar2=None,
                                    op0=Alu.mult)
            nc.vector.tensor_tensor(wws, wws,
                                    bcast(writeout_c, [128, TPB, NS]),
                                    Alu.add)
            cprod = small_pool.tile([128, TPB, 16], dt.float32, tag="cprod")
            nc.vector.tensor_tensor(
                cprod[:].rearrange("p t (j n) -> p t j n", n=NS),
                Ev.rearrange("p t (n j) -> p t j n", j=NS),
                rw[:].unsqueeze(2).broadcast_to([128, TPB, NS, NS]),
                Alu.mult)
            nc.vector.tensor_reduce(
                c_all[:, g * TPB:(g + 1) * TPB, :],
                cprod[:].rearrange("p t (j n) -> p t j n", n=NS),
                Axis.X, Alu.add)
            # shuffle E into (j, b') partition layout for the P7 stationaries
            Ej = E_all[:].rearrange("p t (n j) -> p t n j", j=NS)
            t0 = g * TPB
            for j in range(NS):
                for gg in range(4):
                    nc.gpsimd.dma_start(
                        E_grp[j * 32:(j + 1) * 32, gg, t0:t0 + TPB, :],
                        Ej[gg * 32:(gg + 1) * 32, t0:t0 + TPB, :, j]
                        .unsqueeze(1))

        def p5_tile(t):
            x_bf = x_bfs.pop(t)
            # (j,b')-grouped copies of x for the P7 grouped matmuls,
            # issued early on the idle gpsimd queue to overlap P5/P6
            xgs = []
            for g in range(4):
                xg = xg_pool.tile([128, EMB], dt.bfloat16, tag="xg",
                                  name=f"xg{t}_{g}")
                xgs.append(xg)
                for j in range(NS):
                    nc.gpsimd.dma_start(
                        xg[j * 32:(j + 1) * 32, :],
                        x_bf[g * 32:(g + 1) * 32, j * EMB:(j + 1) * EMB])
            # ---- P5: branch = sum_j c_j x_j (TS mults + TT adds, 2x DVE) --
            br = str_pool.tile([128, EMB], dt.bfloat16, tag="br")
            nc.vector.tensor_scalar(
                out=br[:], in0=x_bf[:, 3 * EMB:4 * EMB],
                scalar1=c_all[:, t, 3:4], scalar2=None, op0=Alu.mult)
            for j in (2, 1, 0):
                tmp = str_pool.tile([128, EMB], dt.bfloat16, tag="tmp",
                                    name=f"tmp{t}_{j}")
                nc.scalar.activation(
                    tmp[:], x_bf[:, j * EMB:(j + 1) * EMB], Act.Identity,
                    scale=c_all[:, t, j:j + 1])
                nc.vector.tensor_tensor(br[:], br[:], tmp[:], Alu.add)
            # 16 PE transposes -> 2 batched psum banks -> fp8 brT
            brT = brt_pool.tile([128, 16, 128], dt.float8e4, tag="brT")
            for half in range(2):
                br_ps = ps_br.tile([128, 8, 128], dt.bfloat16, tag="br_ps")
                for h in range(8):
                    hh = half * 8 + h
                    nc.tensor.transpose(
                        br_ps[:, h, :], br[:, hh * 128:(hh + 1) * 128],
                        ident_bf[:])
                if half == 0:
                    nc.scalar.activation(
                        brT[:, 0:8, :], br_ps[:], Act.Copy)
                else:
                    nc.vector.tensor_copy(brT[:, 8:16, :], br_ps[:])
            x_grps[t] = xgs
            brTs[t] = brT

        def p67_tile(t):
            xgs = x_grps.pop(t)
            brT = brTs.pop(t)
            # ---- P6: y = branch @ W_mod.T via fp8 DoubleRow matmuls ----
            y_nb = str_pool.tile([128, EMB], dt.bfloat16, tag="y_nb")
            for eh in range(4):
                y_ps = ps_y.tile([128, 512], dt.float32, tag="y_ps")
                for c in range(8):
                    nc.tensor.matmul(
                        y_ps[:], brT[:, 2 * c:2 * c + 2, :],
                        wmodT[:, 2 * c:2 * c + 2,
                              eh * 512:(eh + 1) * 512],
                        start=(c == 0), stop=(c == 7), perf_mode=DR)
                if eh % 2 == 0:
                    nc.scalar.activation(y_nb[:, eh * 512:(eh + 1) * 512],
                                         y_ps[:], Act.Copy)
                else:
                    nc.vector.tensor_copy(y_nb[:, eh * 512:(eh + 1) * 512],
                                          y_ps[:])
            # ---- P7: out = E x + diag(ww) y via grouped PE matmuls ----
            # Per 32-row group g: stationary Eblk[(j,b'),(n,b'')] =
            # E_nj[32g+b'] * delta_{b'b''} contracts all 4 streams for 32
            # rows at full PE width; the ww_n y term accumulates into the
            # same psum via a K=32 matmul at partition offset 32g.
            jview = jpat[:].rearrange("p (n c) -> p n c", c=32)
            wwblk = blk_pool.tile([128, 128], dt.bfloat16, tag="wwblk")
            nc.vector.tensor_tensor(
                wwblk[:].rearrange("p (n c) -> p n c", c=32), jview,
                ww_all[:, t, :].unsqueeze(2).broadcast_to([128, NS, 32]),
                Alu.mult)
            eblks = []
            for g in range(4):
                eblk = blk_pool.tile([128, 128], dt.bfloat16, tag="eblk",
                                     name=f"eblk{t}_{g}")
                eblks.append(eblk)
                nc.vector.tensor_tensor(
                    eblk[:].rearrange("p (n c) -> p n c", c=32), jview,
                    E_grp[:, g, t, :].unsqueeze(2)
                    .broadcast_to([128, NS, 32]),
                    Alu.mult)
            for g in range(4):
                gp = slice(g * 32, (g + 1) * 32)
                xg = xgs[g]
                ou = out_pool.tile([128, EMB], dt.float32, tag="ou")
                for k in range(4):
                    kc = slice(k * 512, (k + 1) * 512)
                    o_ps = ps_out.tile([128, 512], dt.float32, tag="y_ps")
                    nc.tensor.matmul(o_ps[:], eblks[g][:], xg[:, kc],
                                     start=True, stop=False)
                    nc.tensor.matmul(o_ps[:], wwblk[gp, :], y_nb[gp, kc],
                                     start=False, stop=True,
                                     tile_position=(g * 32, 0))
                    if (g + k) % 2 == 0:
                        nc.vector.tensor_copy(ou[:, kc], o_ps[:])
                    else:
                        nc.scalar.activation(ou[:, kc], o_ps[:], Act.Copy)
                r0 = t * 128 + g * 32
                nc.sync.dma_start(
                    out_ext[r0:r0 + 32, :, :].transpose([1, 0, 2]), ou[:])

        # ---- schedule ----
        # Block 0: interleave per-chunk loads with P3 transposes+matmuls.
        p3_start(0)
        for q in range(4):
            for i in range(TPB):
                p1_chunk(i, q)
            p3_part(0, q * 16, (q + 1) * 16)
        for i in range(TPB):
            p1_finish(i)
        p3_finish(0)
        p4_smalls(0)
        if NBLK > 1:
            for i in range(TPB - 1):
                p5_tile(i)
                p67_tile(i)
                p1_tile(TPB + i)
            p5_tile(TPB - 1)
            p1_tile(2 * TPB - 1)
            # block 1's P3/P4 overlap tile TPB-1's P6/P7 on the PE
            p3_proj(1)
            p4_smalls(1)
            p67_tile(TPB - 1)
            # software-pipeline block 1 by one tile: each tile's P5/brT
            # completes while the previous tile's P6/P7 occupies the PE
            p5_tile(TPB)
            for i in range(1, TPB):
                p5_tile(TPB + i)
                p67_tile(TPB + i - 1)
            p67_tile(2 * TPB - 1)
        else:
            for i in range(TPB):
                p5_tile(i)
                p67_tile(i)

    nc.compile()
    return nc


def _prep_weights(inputs):
    W_conv = np.asarray(inputs["W_conv"], np.float32)
    W_diss = np.asarray(inputs["W_diss"], np.float32)
    W_dtc = np.asarray(inputs["W_dtc"], np.float32)
    W_dtd = np.asarray(inputs["W_dtd"], np.float32)
    W_read = np.asarray(inputs["W_read"], np.float32)
    W_write = np.asarray(inputs["W_write"], np.float32)
    W_mod = np.asarray(inputs["W_mod"], np.float32)

    Wcat = np.concatenate([W_conv, W_diss, W_dtc, W_dtd, W_read, W_write],
                          axis=0)
    assert Wcat.shape == (NPROJ, IN_DIM)
    wcatT = np.ascontiguousarray(
        Wcat.T.reshape(IN_DIM // 128, 128, NPROJ).transpose(1, 0, 2)
    ).astype(BF16)
    # [k-within-chunk, c, e]: element [p,c,e] = W_mod.T[c*128+p, e]
    wmodT = np.ascontiguousarray(
        W_mod.T.reshape(16, 128, EMB).transpose(1, 0, 2)
    ).astype(ml_dtypes.float8_e4m3)

    scal = dict(
        bias_c=float(np.asarray(inputs["log_dt_c"]).reshape(-1)[0]
                     + np.asarray(inputs["b_dtc"]).reshape(-1)[0]),
        bias_d=float(np.asarray(inputs["log_dt_d"]).reshape(-1)[0]
                     + np.asarray(inputs["b_dtd"]).reshape(-1)[0]),
        alpha_r=float(np.asarray(inputs["alpha_read_in"]).reshape(-1)[0]),
        alpha_w=float(np.asarray(inputs["alpha_write_out"]).reshape(-1)[0]),
    )

    cM = np.asarray(inputs["conserv_A"], np.float32) + \
        np.asarray(inputs["b_conv"], np.float32).reshape(NS, NS)
    skew_const = (cM - cM.T).reshape(-1)
    dissC = (np.asarray(inputs["diss_A"], np.float32) +
             np.asarray(inputs["b_diss"], np.float32).reshape(NS, NS)
             ).reshape(-1)
    eye16 = np.eye(NS, dtype=np.float32).reshape(-1)
    readin = np.asarray(inputs["read_in"], np.float32).reshape(-1)
    writeout = np.asarray(inputs["write_out"], np.float32).reshape(-1)
    cpack = np.concatenate([
        skew_const, dissC, eye16, readin, writeout,
        np.array([scal["bias_c"], scal["bias_d"]], np.float32)]
    ).astype(np.float32)
    assert cpack.shape == (58,)
    # block pattern for P7 stationaries: jpat[p, c] = (p % 32 == c % 32)
    pp = np.arange(128)
    jpat = (pp[:, None] % 32 == pp[None, :] % 32).astype(BF16)
    return wcatT, wmodT, cpack, jpat, scal


_NC_CACHE = {}


def kernel(**inputs):
    from concourse.bass_utils import run_bass_kernel_spmd

    x = np.asarray(inputs["x"], np.float32)
    B = x.shape[0]
    B_loc = B // N_CORES
    wcatT, wmodT, cpack, jpat, scal = _prep_weights(inputs)

    key = (B_loc, tuple(sorted(scal.items())))
    if key not in _NC_CACHE:
        _NC_CACHE[key] = _build(B_loc, scal)
    nc = _NC_CACHE[key]

    xf = x.reshape(B, IN_DIM)
    in_maps = []
    for i in range(N_CORES):
        in_maps.append({
            "x": np.ascontiguousarray(xf[i * B_loc:(i + 1) * B_loc]),
            "wcatT": wcatT,
            "wmodT": wmodT,
            "cpack": cpack,
            "jpat": jpat,
        })

    trace = os.environ.get("KERNEL_TRACE", "0") == "1"
    res = run_bass_kernel_spmd(nc, in_maps, core_ids=list(range(N_CORES)),
                               trace=trace)
    if trace and res.exec_time_ns is not None:
        print(f"HW exec time: {res.exec_time_ns} ns")
        kernel.last_exec_time_ns = res.exec_time_ns
    out = np.concatenate([res.results[i]["out"] for i in range(N_CORES)],
                         axis=0)
    return out


# revision 54
# speedup vs baseline: 1.0062x; 1.0062x over previous
"""Trainium2 Bass kernel for nn_ContinuousGenHyperConnections.

Sharding: data-parallel over the batch dim B=8192 across 8 NeuronCores
(1024 rows each). All weights replicated; no collectives.

Per-core dataflow (B_loc=1024 -> 8 b-tiles of 128 rows; proj in blocks of
TPB=4 tiles; block 0's chunk loads are interleaved with its P3 chunks):
  P1 per tile : DMA x fp32 chunks (scalar queue); cast->bf16 into resident
                x_bf (DVE/ACT); sum-of-squares via ACT Square accum + DVE
                STT accum; s = 1/sqrt(mean+eps).
  P3 per block: x transposed ON-CHIP: PE transposes 2 chunks x 4 tiles
                into a bf16 psum bank, one [128,2,512] copy out, then
                bf16 matmuls accumulate proj.T [42,512]; PE-transpose
                + scale by s -> proj_all (no DRAM round-trip).
  P4 per block: per-row 4x4 generator math batched over 4 tiles (batched
                4x4 matmuls on DVE, order-3 Taylor expm + 4 squarings),
                rw/ww, c = E^T rw; E shuffled to (j,b') partition layout
                (E_grp) via tiny gpsimd DMAs.
  P5 per tile : branch = sum_j c_j x_j (ACT mults + DVE adds),
                16 PE-transposes -> brT fp8 (batched psum copies).
  P6 per tile : y = branch @ W_mod.T via fp8 DoubleRow matmuls
                (2 k-chunks per instr at 0.5 cyc/row) -> y_nb [128,2048].
  P7 per tile : out = E x + diag(ww) y on the PE: x is regrouped per
                32-row group g to partitions (j,b') (16 gpsimd SBUF-SBUF
                DMAs/tile, issued early); stationary Eblk[(j,b'),(n,b'')]
                = E_nj[b]*delta builds via one jpat TT per group; a K=128
                matmul mixes all 4 streams for 32 rows at full PE width,
                and a K=32 matmul at partition offset 32g adds ww_n y into
                the same psum; psum -> fp32 out copies alternate DVE/ACT;
                out DMA (sync queue) uses a transposed 3D DRAM AP.
"""

import os
import sys

sys.path.insert(0, "/opt/trn_rl_repo")

import numpy as np
import ml_dtypes

BF16 = ml_dtypes.bfloat16

DT_MIN, DT_MAX = 1e-3, 1.0
EPS = 1e-6
NS = 4  # streams
EMB = 2048
IN_DIM = 8192
N_CORES = 8
NPROJ = 42  # 16 conv + 16 diss + 1 dtc + 1 dtd + 4 read + 4 write


def _build(B_loc, scal, num_devices=N_CORES):
    import concourse.bacc as bacc
    import concourse.mybir as mybir
    import concourse.tile as tile
    from concourse.masks import make_identity
    from contextlib import ExitStack

    dt = mybir.dt
    Alu = mybir.AluOpType
    Act = mybir.ActivationFunctionType
    Axis = mybir.AxisListType
    DR = mybir.MatmulPerfMode.DoubleRow

    NT = B_loc // 128
    TPB = min(4, NT)          # tiles per proj block
    NBLK = NT // TPB
    NCH = IN_DIM // 128       # 64 contraction chunks
    NB = TPB * 128            # rows per proj block

    # expm 2^-4 prescale folded into dt: dt_eff = (DT_MIN + range*sig)/16
    R_SIG = (DT_MAX - DT_MIN) / 16.0
    C_SIG = DT_MIN / 16.0

    nc = bacc.Bacc("TRN2", target_bir_lowering=False, debug=False,
                   num_devices=num_devices)

    x_ext = nc.declare_dram_parameter("x", [B_loc, IN_DIM], dt.float32,
                                      isOutput=False)
    wcatT_ext = nc.declare_dram_parameter("wcatT", [128, NCH, NPROJ],
                                          dt.bfloat16, isOutput=False)
    wmodT_ext = nc.declare_dram_parameter("wmodT", [128, 16, EMB],
                                          dt.float8e4, isOutput=False)
    cpack_ext = nc.declare_dram_parameter("cpack", [58], dt.float32,
                                          isOutput=False)
    jpat_ext = nc.declare_dram_parameter("jpat", [128, 128], dt.bfloat16,
                                         isOutput=False)
    out_ext = nc.declare_dram_parameter("out", [B_loc, NS, EMB], dt.float32,
                                        isOutput=True)

    with tile.TileContext(nc) as tc, ExitStack() as ctx:
        const_pool = ctx.enter_context(tc.tile_pool(name="const", bufs=1))
        p1_pool = ctx.enter_context(tc.tile_pool(name="p1", bufs=2))
        xbb_pool = ctx.enter_context(tc.tile_pool(name="xbb", bufs=4))
        xt_pool = ctx.enter_context(tc.tile_pool(name="xt", bufs=2))
        small_pool = ctx.enter_context(tc.tile_pool(name="small", bufs=2))
        sm1_pool = ctx.enter_context(tc.tile_pool(name="sm1", bufs=1))
        str_pool = ctx.enter_context(tc.tile_pool(name="stream", bufs=2))
        brt_pool = ctx.enter_context(tc.tile_pool(name="brt", bufs=2))
        out_pool = ctx.enter_context(tc.tile_pool(name="outp", bufs=2))
        xg_pool = ctx.enter_context(tc.tile_pool(name="xg", bufs=6))
        blk_pool = ctx.enter_context(tc.tile_pool(name="blk", bufs=5))
        ps_proj = ctx.enter_context(
            tc.tile_pool(name="ps_proj", bufs=1, space="PSUM"))
        ps_trp = ctx.enter_context(
            tc.tile_pool(name="ps_trp", bufs=1, space="PSUM"))
        ps_br = ctx.enter_context(
            tc.tile_pool(name="ps_br", bufs=2, space="PSUM"))
        ps_y = ctx.enter_context(
            tc.tile_pool(name="ps_y", bufs=4, space="PSUM"))
        ps_out = ps_y

        # ---- constants ----
        wcatT = const_pool.tile([128, NCH, NPROJ], dt.bfloat16)
        nc.sync.dma_start(wcatT[:], wcatT_ext[:])
        wmodT = const_pool.tile([128, 16, EMB], dt.float8e4)
        nc.scalar.dma_start(wmodT[:], wmodT_ext[:])
        cpk = const_pool.tile([128, 58], dt.float32)
        nc.sync.dma_start(cpk[:], cpack_ext[:].partition_broadcast(128))
        jpat = const_pool.tile([128, 128], dt.bfloat16)
        nc.sync.dma_start(jpat[:], jpat_ext[:])
        ident_bf = const_pool.tile([128, 128], dt.bfloat16)
        make_identity(nc, ident_bf[:])
        ident_f32 = const_pool.tile([128, 128], dt.float32)
        make_identity(nc, ident_f32[:])

        skew_c = cpk[:, 0:16]     # (conservA+bconv) - transpose, flattened
        diss_c = cpk[:, 16:32]    # dissA + bdiss, flattened
        eye16 = cpk[:, 32:48]     # flattened I4
        readin_c = cpk[:, 48:52]
        writeout_c = cpk[:, 52:56]

        s_all = sm1_pool.tile([128, NT], dt.float32)
        proj_all = sm1_pool.tile([128, NT, NPROJ], dt.float32)
        E_all = sm1_pool.tile([128, NT, 16], dt.float32)
        E_grp = sm1_pool.tile([128, 4, NT, NS], dt.float32)
        c_all = sm1_pool.tile([128, NT, NS], dt.float32)
        ww_all = sm1_pool.tile([128, NT, NS], dt.float32)
        ss_all = sm1_pool.tile([128, NT, 4], dt.float32)

        def bcast(ap2d, shape):
            return ap2d.unsqueeze(1).broadcast_to(shape)

        x_bfs = {}

        x_grps = {}
        brTs = {}

        p1_state = {}

        def p1_chunk(t, q):
            """load + cast + sum-of-squares for chunk q of tile t."""
            if q == 0:
                x_bf = xbb_pool.tile([128, IN_DIM], dt.bfloat16,
                                     tag="x_bf", name=f"x_bf{t}")
                x_bfs[t] = x_bf
            x_bf = x_bfs[t]
            ss = ss_all[:, t, :]
            xf = p1_pool.tile([128, EMB], dt.float32, tag="xf")
            eng = nc.sync if (t < TPB and q % 2 == 0) else nc.scalar
            eng.dma_start(
                xf[:], x_ext[t * 128:(t + 1) * 128,
                             q * EMB:(q + 1) * EMB])
            if q % 2 == 0:
                nc.vector.tensor_copy(x_bf[:, q * EMB:(q + 1) * EMB],
                                      xf[:])
                sqj = str_pool.tile([128, EMB], dt.bfloat16, tag="tmp")
                nc.scalar.activation(sqj[:], xf[:], Act.Square,
                                     accum_out=ss[:, q:q + 1])
            else:
                nc.scalar.activation(x_bf[:, q * EMB:(q + 1) * EMB],
                                     xf[:], Act.Copy)
                sqj = str_pool.tile([128, EMB], dt.bfloat16, tag="tmp")
                nc.vector.scalar_tensor_tensor(
                    out=sqj[:], in0=x_bf[:, q * EMB:(q + 1) * EMB],
                    scalar=1.0, in1=x_bf[:, q * EMB:(q + 1) * EMB],
                    op0=Alu.bypass, op1=Alu.mult,
                    accum_out=ss[:, q:q + 1])

        def p1_finish(t):
            ssum = small_pool.tile([128, 1], dt.float32, tag="s01")
            nc.vector.tensor_reduce(ssum[:], ss_all[:, t, :], Axis.X,
                                    Alu.add)
            nc.vector.tensor_scalar(
                out=ssum[:], in0=ssum[:], scalar1=1.0 / IN_DIM,
                scalar2=EPS, op0=Alu.mult, op1=Alu.add)
            sqr = small_pool.tile([128, 1], dt.float32, tag="sqr")
            nc.scalar.activation(sqr[:], ssum[:], Act.Sqrt)
            nc.vector.reciprocal(s_all[:, t:t + 1], sqr[:])

        def p1_tile(t):
            for q in range(4):
                p1_chunk(t, q)
            p1_finish(t)

        p3_state = {}

        def p3_start(g):
            p3_state[g] = ps_proj.tile([NPROJ, NB], dt.float32,
                                       tag="proj_ps", name=f"proj_ps{g}")

        def p3_part(g, c0, c1):
            """proj.T chunks [c0,c1) via on-chip PE transposes + matmul."""
            proj_ps = p3_state[g]
            for c2 in range(c0 // 2, c1 // 2):
                # transpose 2 chunks x 4 tiles into one bf16 psum bank
                tp = ps_br.tile([128, 8, 128], dt.bfloat16, tag="br_ps",
                                name=f"xtp{g}_{c2}")
                for cc in (2 * c2, 2 * c2 + 1):
                    for i in range(TPB):
                        nc.tensor.transpose(
                            tp[:, (cc % 2) * TPB + i, :],
                            x_bfs[g * TPB + i][:, cc * 128:(cc + 1) * 128],
                            ident_bf[:])
                xt = xt_pool.tile([128, 2, NB], dt.bfloat16, tag="xt")
                if c2 % 2 == 0:
                    nc.vector.tensor_copy(xt[:], tp[:])
                else:
                    nc.scalar.activation(xt[:], tp[:], Act.Copy)
                for cc in (2 * c2, 2 * c2 + 1):
                    nc.tensor.matmul(proj_ps[:], wcatT[:, cc, :],
                                     xt[:, cc % 2, :],
                                     start=(cc == 0), stop=(cc == NCH - 1))

        def p3_finish(g):
            proj_ps = p3_state.pop(g)
            projT = sm1_pool.tile([NPROJ, NB], dt.float32, tag="projT")
            nc.vector.tensor_copy(projT[:], proj_ps[:])
            for i in range(TPB):
                t = g * TPB + i
                tr_ps = ps_trp.tile([128, NPROJ], dt.float32, tag="tr_ps")
                nc.tensor.transpose(
                    tr_ps[:], projT[:, i * 128:(i + 1) * 128],
                    ident_f32[:NPROJ, :NPROJ])
                nc.vector.tensor_scalar(
                    out=proj_all[:, t, :], in0=tr_ps[:],
                    scalar1=s_all[:, t:t + 1], scalar2=None, op0=Alu.mult)

        def p3_proj(g):
            p3_start(g)
            p3_part(g, 0, NCH)
            p3_finish(g)

        def p4_smalls(g):
            """per-row generator math for block g, batched over TPB tiles."""
            pb = proj_all[:, g * TPB:(g + 1) * TPB, :]   # [128,TPB,42]

            def mm2(dst, lhs, rhs, rhs_pat):
                # batched per-row 4x4 matmul: loop j (broadcast dim) only
                pr = small_pool.tile([128, TPB, 64], dt.float32, tag="prod")
                pv5 = pr[:].rearrange("p t (i j k) -> p t i j k",
                                      j=NS, k=NS)
                lv = lhs[:].rearrange("p t (i k) -> p t i k", k=NS)
                rv = rhs[:].rearrange(rhs_pat, j=NS)
                for j in range(NS):
                    nc.vector.tensor_tensor(
                        pv5[:, :, :, j, :], lv,
                        rv[:, :, j, :].unsqueeze(2)
                        .broadcast_to([128, TPB, NS, NS]),
                        Alu.mult)
                nc.vector.tensor_reduce(
                    dst[:], pr[:].rearrange("p t (ij k) -> p t ij k", k=NS),
                    Axis.X, Alu.add)

            smw = small_pool.tile([128, TPB, 16], dt.float32, tag="smw")
            nc.vector.tensor_tensor(
                smw[:].rearrange("p t (i j) -> p t i j", j=NS),
                pb[:, :, 0:16].rearrange("p t (i j) -> p t i j", j=NS),
                pb[:, :, 0:16].rearrange("p t (j i) -> p t i j", i=NS),
                Alu.subtract)
            nc.vector.tensor_tensor(smw[:], smw[:],
                                    bcast(skew_c, [128, TPB, 16]), Alu.add)
            Rm = small_pool.tile([128, TPB, 16], dt.float32, tag="Rm")
            nc.vector.tensor_tensor(Rm[:], pb[:, :, 16:32],
                                    bcast(diss_c, [128, TPB, 16]), Alu.add)
            dtc = small_pool.tile([128, TPB, 1], dt.float32, tag="dtc")
            dtd = small_pool.tile([128, TPB, 1], dt.float32, tag="dtd")
            nc.scalar.activation(dtc[:], pb[:, :, 32:33], Act.Sigmoid,
                                 bias=cpk[:, 56:57])
            nc.scalar.activation(dtd[:], pb[:, :, 33:34], Act.Sigmoid,
                                 bias=cpk[:, 57:58])
            nc.vector.tensor_scalar(out=dtc[:], in0=dtc[:], scalar1=R_SIG,
                                    scalar2=C_SIG, op0=Alu.mult, op1=Alu.add)
            nc.vector.tensor_scalar(out=dtd[:], in0=dtd[:], scalar1=R_SIG,
                                    scalar2=C_SIG, op0=Alu.mult, op1=Alu.add)

            # K = R @ R^T
            Km = small_pool.tile([128, TPB, 16], dt.float32, tag="Km")
            mm2(Km, Rm, Rm, "p t (j k) -> p t j k")
            # A = dtc*skew - dtd*K  (batched: broadcast dt over the 16 cols)
            Am = small_pool.tile([128, TPB, 16], dt.float32, tag="Am")
            tmpA = small_pool.tile([128, TPB, 16], dt.float32, tag="tmpA")
            nc.vector.tensor_tensor(
                Am[:], smw[:], dtc[:].broadcast_to([128, TPB, 16]), Alu.mult)
            nc.vector.tensor_tensor(
                tmpA[:], Km[:], dtd[:].broadcast_to([128, TPB, 16]),
                Alu.mult)
            nc.vector.tensor_tensor(Am[:], Am[:], tmpA[:], Alu.subtract)
            # expm: order-4 Taylor (||A/16|| <= ~1.1 -> err < 2e-4) + 4 sq
            A2 = small_pool.tile([128, TPB, 16], dt.float32, tag="A2")
            A3 = small_pool.tile([128, TPB, 16], dt.float32, tag="A3")
            mm2(A2, Am, Am, "p t (k j) -> p t j k")
            mm2(A3, A2, Am, "p t (k j) -> p t j k")
            Em = small_pool.tile([128, TPB, 16], dt.float32, tag="Em")
            nc.vector.tensor_tensor(Em[:], Am[:],
                                    bcast(eye16, [128, TPB, 16]), Alu.add)
            nc.vector.scalar_tensor_tensor(
                out=Em[:], in0=A2[:], scalar=0.5, in1=Em[:],
                op0=Alu.mult, op1=Alu.add)
            nc.vector.scalar_tensor_tensor(
                out=Em[:], in0=A3[:], scalar=1.0 / 6.0, in1=Em[:],
                op0=Alu.mult, op1=Alu.add)
            E2 = small_pool.tile([128, TPB, 16], dt.float32, tag="E2")
            cur, nxt = Em, E2
            for q in range(4):
                if q == 3:
                    mm2(E_all[:, g * TPB:(g + 1) * TPB, :], cur, cur,
                        "p t (k j) -> p t j k")
                else:
                    mm2(nxt, cur, cur, "p t (k j) -> p t j k")
                    cur, nxt = nxt, cur
            Ev = E_all[:, g * TPB:(g + 1) * TPB, :]
            # rw / ww / c
            rw = small_pool.tile([128, TPB, NS], dt.float32, tag="rw")
            nc.vector.tensor_scalar(out=rw[:], in0=pb[:, :, 34:38],
                                    scalar1=scal["alpha_r"], scalar2=None,
                                    op0=Alu.mult)
            nc.vector.tensor_tensor(rw[:], rw[:],
                                    bcast(readin_c, [128, TPB, NS]), Alu.add)
            nc.scalar.activation(rw[:], rw[:], Act.Sigmoid)
            wws = ww_all[:, g * TPB:(g + 1) * TPB, :]
            nc.vector.tensor_scalar(out=wws, in0=pb[:, :, 38:42],
                                    scalar1=scal["alpha_w"], scalar2=None,
                                    op0=Alu.mult)
            nc.vector.tensor_tensor(wws, wws,
                                    bcast(writeout_c, [128, TPB, NS]),
                                    Alu.add)
            cprod = small_pool.tile([128, TPB, 16], dt.float32, tag="cprod")
            nc.vector.tensor_tensor(
                cprod[:].rearrange("p t (j n) -> p t j n", n=NS),
                Ev.rearrange("p t (n j) -> p t j n", j=NS),
                rw[:].unsqueeze(2).broadcast_to([128, TPB, NS, NS]),
                Alu.mult)
            nc.vector.tensor_reduce(
                c_all[:, g * TPB:(g + 1) * TPB, :],
                cprod[:].rearrange("p t (j n) -> p t j n", n=NS),
                Axis.X, Alu.add)
            # shuffle E into (j, b') partition layout for the P7 stationaries
            Ej = E_all[:].rearrange("p t (n j) -> p t n j", j=NS)
            t0 = g * TPB
            for j in range(NS):
                for gg in range(4):
                    nc.gpsimd.dma_start(
                        E_grp[j * 32:(j + 1) * 32, gg, t0:t0 + TPB, :],
                        Ej[gg * 32:(gg + 1) * 32, t0:t0 + TPB, :, j]
                        .unsqueeze(1))

        def xg_shuffle(t):
            # (j,b')-grouped copies of x for the P7 grouped matmuls,
            # issued on the idle gpsimd queue
            x_bf = x_bfs.pop(t)
            xgs = []
            for g in range(4):
                xg = xg_pool.tile([128, EMB], dt.bfloat16, tag="xg",
                                  name=f"xg{t}_{g}")
                xgs.append(xg)
                for j in range(NS):
                    nc.gpsimd.dma_start(
                        xg[j * 32:(j + 1) * 32, :],
                        x_bf[g * 32:(g + 1) * 32, j * EMB:(j + 1) * EMB])
            x_grps[t] = xgs

        def p5_tile(t, shuf_early=True):
            x_bf = x_bfs[t]
            if shuf_early:
                xg_shuffle(t)
            # ---- P5: branch = sum_j c_j x_j (TS mults + TT adds, 2x DVE) --
            br = str_pool.tile([128, EMB], dt.bfloat16, tag="br")
            nc.vector.tensor_scalar(
                out=br[:], in0=x_bf[:, 3 * EMB:4 * EMB],
                scalar1=c_all[:, t, 3:4], scalar2=None, op0=Alu.mult)
            for j in (2, 1, 0):
                tmp = str_pool.tile([128, EMB], dt.bfloat16, tag="tmp",
                                    name=f"tmp{t}_{j}")
                nc.scalar.activation(
                    tmp[:], x_bf[:, j * EMB:(j + 1) * EMB], Act.Identity,
                    scale=c_all[:, t, j:j + 1])
                nc.vector.tensor_tensor(br[:], br[:], tmp[:], Alu.add)
            # 16 PE transposes -> 2 batched psum banks -> fp8 brT
            brT = brt_pool.tile([128, 16, 128], dt.float8e4, tag="brT")
            for half in range(2):
                br_ps = ps_br.tile([128, 8, 128], dt.bfloat16, tag="br_ps")
                for h in range(8):
                    hh = half * 8 + h
                    nc.tensor.transpose(
                        br_ps[:, h, :], br[:, hh * 128:(hh + 1) * 128],
                        ident_bf[:])
                if half == 0:
                    nc.scalar.activation(
                        brT[:, 0:8, :], br_ps[:], Act.Copy)
                else:
                    nc.vector.tensor_copy(brT[:, 8:16, :], br_ps[:])
            brTs[t] = brT

        def p67_tile(t):
            if t not in x_grps:
                xg_shuffle(t)
            xgs = x_grps.pop(t)
            brT = brTs.pop(t)
            # ---- P6: y = branch @ W_mod.T via fp8 DoubleRow matmuls ----
            y_nb = str_pool.tile([128, EMB], dt.bfloat16, tag="y_nb")
            for eh in range(4):
                y_ps = ps_y.tile([128, 512], dt.float32, tag="y_ps")
                for c in range(8):
                    nc.tensor.matmul(
                        y_ps[:], brT[:, 2 * c:2 * c + 2, :],
                        wmodT[:, 2 * c:2 * c + 2,
                              eh * 512:(eh + 1) * 512],
                        start=(c == 0), stop=(c == 7), perf_mode=DR)
                if eh % 2 == 0:
                    nc.scalar.activation(y_nb[:, eh * 512:(eh + 1) * 512],
                                         y_ps[:], Act.Copy)
                else:
                    nc.vector.tensor_copy(y_nb[:, eh * 512:(eh + 1) * 512],
                                          y_ps[:])
            # ---- P7: out = E x + diag(ww) y via grouped PE matmuls ----
            # Per 32-row group g: stationary Eblk[(j,b'),(n,b'')] =
            # E_nj[32g+b'] * delta_{b'b''} contracts all 4 streams for 32
            # rows at full PE width; the ww_n y term accumulates into the
            # same psum via a K=32 matmul at partition offset 32g.
            jview = jpat[:].rearrange("p (n c) -> p n c", c=32)
            wwblk = blk_pool.tile([128, 128], dt.bfloat16, tag="wwblk")
            nc.vector.tensor_tensor(
                wwblk[:].rearrange("p (n c) -> p n c", c=32), jview,
                ww_all[:, t, :].unsqueeze(2).broadcast_to([128, NS, 32]),
                Alu.mult)
            eblks = []
            for g in range(4):
                eblk = blk_pool.tile([128, 128], dt.bfloat16, tag="eblk",
                                     name=f"eblk{t}_{g}")
                eblks.append(eblk)
                nc.vector.tensor_tensor(
                    eblk[:].rearrange("p (n c) -> p n c", c=32), jview,
                    E_grp[:, g, t, :].unsqueeze(2)
                    .broadcast_to([128, NS, 32]),
                    Alu.mult)
            for g in range(4):
                gp = slice(g * 32, (g + 1) * 32)
                xg = xgs[g]
                ou = out_pool.tile([128, EMB], dt.float32, tag="ou")
                for k in range(4):
                    kc = slice(k * 512, (k + 1) * 512)
                    o_ps = ps_out.tile([128, 512], dt.float32, tag="y_ps")
                    nc.tensor.matmul(o_ps[:], eblks[g][:], xg[:, kc],
                                     start=True, stop=False)
                    nc.tensor.matmul(o_ps[:], wwblk[gp, :], y_nb[gp, kc],
                                     start=False, stop=True,
                                     tile_position=(g * 32, 0))
                    if (g + k) % 2 == 0:
                        nc.vector.tensor_copy(ou[:, kc], o_ps[:])
                    else:
                        nc.scalar.activation(ou[:, kc], o_ps[:], Act.Copy)
                r0 = t * 128 + g * 32
                nc.sync.dma_start(
                    out_ext[r0:r0 + 32, :, :].transpose([1, 0, 2]), ou[:])

        # ---- schedule ----
        # Block 0: interleave per-chunk loads with P3 transposes+matmuls.
        p3_start(0)
        for q in range(4):
            for i in range(TPB):
                p1_chunk(i, q)
            p3_part(0, q * 16, (q + 1) * 16)
        for i in range(TPB):
            p1_finish(i)
        p3_finish(0)
        p4_smalls(0)
        if NBLK > 1:
            for i in range(TPB - 1):
                p5_tile(i)
                p67_tile(i)
                p1_tile(TPB + i)
            p5_tile(TPB - 1)
            p1_tile(2 * TPB - 1)
            # block 1's P3/P4 overlap tile TPB-1's P6/P7 on the PE
            p3_proj(1)
            p4_smalls(1)
            p67_tile(TPB - 1)
            # pipeline block 1 by one tile: P5/brT of tile t+1 run while
            # tile t's P6/P7 holds the PE; xg shuffles defer to p67 so the
            # xg pool only ever holds one tile's groups
            p5_tile(TPB, shuf_early=False)
            for i in range(1, TPB):
                p5_tile(TPB + i, shuf_early=False)
                p67_tile(TPB + i - 1)
            p67_tile(2 * TPB - 1)
        else:
            for i in range(TPB):
                p5_tile(i)
                p67_tile(i)

    nc.compile()
    return nc


def _prep_weights(inputs):
    W_conv = np.asarray(inputs["W_conv"], np.float32)
    W_diss = np.asarray(inputs["W_diss"], np.float32)
    W_dtc = np.asarray(inputs["W_dtc"], np.float32)
    W_dtd = np.asarray(inputs["W_dtd"], np.float32)
    W_read = np.asarray(inputs["W_read"], np.float32)
    W_write = np.asarray(inputs["W_write"], np.float32)
    W_mod = np.asarray(inputs["W_mod"], np.float32)

    Wcat = np.concatenate([W_conv, W_diss, W_dtc, W_dtd, W_read, W_write],
                          axis=0)
    assert Wcat.shape == (NPROJ, IN_DIM)
    wcatT = np.ascontiguousarray(
        Wcat.T.reshape(IN_DIM // 128, 128, NPROJ).transpose(1, 0, 2)
    ).astype(BF16)
    # [k-within-chunk, c, e]: element [p,c,e] = W_mod.T[c*128+p, e]
    wmodT = np.ascontiguousarray(
        W_mod.T.reshape(16, 128, EMB).transpose(1, 0, 2)
    ).astype(ml_dtypes.float8_e4m3)

    scal = dict(
        bias_c=float(np.asarray(inputs["log_dt_c"]).reshape(-1)[0]
                     + np.asarray(inputs["b_dtc"]).reshape(-1)[0]),
        bias_d=float(np.asarray(inputs["log_dt_d"]).reshape(-1)[0]
                     + np.asarray(inputs["b_dtd"]).reshape(-1)[0]),
        alpha_r=float(np.asarray(inputs["alpha_read_in"]).reshape(-1)[0]),
        alpha_w=float(np.asarray(inputs["alpha_write_out"]).reshape(-1)[0]),
    )

    cM = np.asarray(inputs["conserv_A"], np.float32) + \
        np.asarray(inputs["b_conv"], np.float32).reshape(NS, NS)
    skew_const = (cM - cM.T).reshape(-1)
    dissC = (np.asarray(inputs["diss_A"], np.float32) +
             np.asarray(inputs["b_diss"], np.float32).reshape(NS, NS)
             ).reshape(-1)
    eye16 = np.eye(NS, dtype=np.float32).reshape(-1)
    readin = np.asarray(inputs["read_in"], np.float32).reshape(-1)
    writeout = np.asarray(inputs["write_out"], np.float32).reshape(-1)
    cpack = np.concatenate([
        skew_const, dissC, eye16, readin, writeout,
        np.array([scal["bias_c"], scal["bias_d"]], np.float32)]
    ).astype(np.float32)
    assert cpack.shape == (58,)
    # block pattern for P7 stationaries: jpat[p, c] = (p % 32 == c % 32)
    pp = np.arange(128)
    jpat = (pp[:, None] % 32 == pp[None, :] % 32).astype(BF16)
    return wcatT, wmodT, cpack, jpat, scal


_NC_CACHE = {}


def kernel(**inputs):
    from concourse.bass_utils import run_bass_kernel_spmd

    x = np.asarray(inputs["x"], np.float32)
    B = x.shape[0]
    B_loc = B // N_CORES
    wcatT, wmodT, cpack, jpat, scal = _prep_weights(inputs)

    key = (B_loc, tuple(sorted(scal.items())))
    if key not in _NC_CACHE:
        _NC_CACHE[key] = _build(B_loc, scal)
    nc = _NC_CACHE[key]

    xf = x.reshape(B, IN_DIM)
    in_maps = []
    for i in range(N_CORES):
        in_maps.append({
            "x": np.ascontiguousarray(xf[i * B_loc:(i + 1) * B_loc]),
            "wcatT": wcatT,
            "wmodT": wmodT,
            "cpack": cpack,
            "jpat": jpat,
        })

    trace = os.environ.get("KERNEL_TRACE", "0") == "1"
    res = run_bass_kernel_spmd(nc, in_maps, core_ids=list(range(N_CORES)),
                               trace=trace)
    if trace and res.exec_time_ns is not None:
        print(f"HW exec time: {res.exec_time_ns} ns")
        kernel.last_exec_time_ns = res.exec_time_ns
    out = np.concatenate([res.results[i]["out"] for i in range(N_CORES)],
                         axis=0)
    return out


# revision 56
# speedup vs baseline: 1.0877x; 1.0810x over previous
"""Trainium2 Bass kernel for nn_ContinuousGenHyperConnections.

Sharding: data-parallel over the batch dim B=8192 across 8 NeuronCores
(1024 rows each). All weights replicated; no collectives.

Per-core dataflow (B_loc=1024 -> 8 b-tiles of 128 rows; proj in blocks of
TPB=4 tiles; block 0's chunk loads are interleaved with its P3 chunks):
  P1 per tile : DMA x fp32 chunks (scalar queue); cast->bf16 into resident
                x_bf (DVE/ACT); sum-of-squares via ACT Square accum + DVE
                STT accum; s = 1/sqrt(mean+eps).
  P3 per block: x transposed ON-CHIP: PE transposes 2 chunks x 4 tiles
                into a bf16 psum bank, one [128,2,512] copy out, then
                bf16 matmuls accumulate proj.T [42,512]; PE-transpose
                + scale by s -> proj_all (no DRAM round-trip).
  P4 per block: per-row 4x4 generator math batched over 4 tiles (batched
                4x4 matmuls on DVE, order-3 Taylor expm + 4 squarings),
                rw/ww, c = E^T rw; E shuffled to (j,b') partition layout
                (E_grp) via tiny gpsimd DMAs.
  P5 per tile : branch = sum_j c_j x_j (ACT mults + DVE adds),
                16 PE-transposes -> brT fp8 (batched psum copies).
  P6 per tile : y = branch @ W_mod.T via fp8 DoubleRow matmuls
                (2 k-chunks per instr at 0.5 cyc/row) -> y_nb [128,2048].
  P7 per tile : out = E x + diag(ww) y on the PE: x is regrouped per
                32-row group g to partitions (j,b') (16 gpsimd SBUF-SBUF
                DMAs/tile, issued early); stationary Eblk[(j,b'),(n,b'')]
                = E_nj[b]*delta builds via one jpat TT per group; a K=128
                matmul mixes all 4 streams for 32 rows at full PE width,
                and a K=32 matmul at partition offset 32g adds ww_n y into
                the same psum; psum -> fp32 out copies alternate DVE/ACT;
                out DMA (sync queue) uses a transposed 3D DRAM AP.
"""

import os
import sys

sys.path.insert(0, "/opt/trn_rl_repo")

import numpy as np
import ml_dtypes

BF16 = ml_dtypes.bfloat16

DT_MIN, DT_MAX = 1e-3, 1.0
EPS = 1e-6
NS = 4  # streams
EMB = 2048
IN_DIM = 8192
N_CORES = 8
NPROJ = 42  # 16 conv + 16 diss + 1 dtc + 1 dtd + 4 read + 4 write


def _build(B_loc, scal, num_devices=N_CORES):
    import concourse.bacc as bacc
    import concourse.mybir as mybir
    import concourse.tile as tile
    from concourse.masks import make_identity
    from contextlib import ExitStack

    dt = mybir.dt
    Alu = mybir.AluOpType
    Act = mybir.ActivationFunctionType
    Axis = mybir.AxisListType
    DR = mybir.MatmulPerfMode.DoubleRow

    NT = B_loc // 128
    TPB = min(4, NT)          # tiles per proj block
    NBLK = NT // TPB
    NCH = IN_DIM // 128       # 64 contraction chunks
    NB = TPB * 128            # rows per proj block

    # expm 2^-4 prescale folded into dt: dt_eff = (DT_MIN + range*sig)/16
    R_SIG = (DT_MAX - DT_MIN) / 16.0
    C_SIG = DT_MIN / 16.0

    nc = bacc.Bacc("TRN2", target_bir_lowering=False, debug=False,
                   num_devices=num_devices)

    x_ext = nc.declare_dram_parameter("x", [B_loc, IN_DIM], dt.float32,
                                      isOutput=False)
    wcatT_ext = nc.declare_dram_parameter("wcatT", [128, NCH, NPROJ],
                                          dt.bfloat16, isOutput=False)
    wmodT_ext = nc.declare_dram_parameter("wmodT", [128, 16, EMB],
                                          dt.float8e4, isOutput=False)
    cpack_ext = nc.declare_dram_parameter("cpack", [58], dt.float32,
                                          isOutput=False)
    jpat_ext = nc.declare_dram_parameter("jpat", [128, 128], dt.bfloat16,
                                         isOutput=False)
    out_ext = nc.declare_dram_parameter("out", [B_loc, NS, EMB], dt.float32,
                                        isOutput=True)

    with tile.TileContext(nc) as tc, ExitStack() as ctx:
        const_pool = ctx.enter_context(tc.tile_pool(name="const", bufs=1))
        p1_pool = ctx.enter_context(tc.tile_pool(name="p1", bufs=2))
        xbb_pool = ctx.enter_context(tc.tile_pool(name="xbb", bufs=4))
        xt_pool = ctx.enter_context(tc.tile_pool(name="xt", bufs=2))
        small_pool = ctx.enter_context(tc.tile_pool(name="small", bufs=2))
        sm1_pool = ctx.enter_context(tc.tile_pool(name="sm1", bufs=1))
        str_pool = ctx.enter_context(tc.tile_pool(name="stream", bufs=2))
        brt_pool = ctx.enter_context(tc.tile_pool(name="brt", bufs=1))
        out_pool = ctx.enter_context(tc.tile_pool(name="outp", bufs=2))
        xg_pool = ctx.enter_context(tc.tile_pool(name="xg", bufs=6))
        blk_pool = ctx.enter_context(tc.tile_pool(name="blk", bufs=5))
        ps_proj = ctx.enter_context(
            tc.tile_pool(name="ps_proj", bufs=1, space="PSUM"))
        ps_trp = ctx.enter_context(
            tc.tile_pool(name="ps_trp", bufs=1, space="PSUM"))
        ps_br = ctx.enter_context(
            tc.tile_pool(name="ps_br", bufs=2, space="PSUM"))
        ps_y = ctx.enter_context(
            tc.tile_pool(name="ps_y", bufs=4, space="PSUM"))
        ps_out = ps_y

        # ---- constants ----
        wcatT = const_pool.tile([128, NCH, NPROJ], dt.bfloat16)
        nc.sync.dma_start(wcatT[:], wcatT_ext[:])
        wmodT = const_pool.tile([128, 16, EMB], dt.float8e4)
        nc.scalar.dma_start(wmodT[:], wmodT_ext[:])
        cpk = const_pool.tile([128, 58], dt.float32)
        nc.sync.dma_start(cpk[:], cpack_ext[:].partition_broadcast(128))
        jpat = const_pool.tile([128, 128], dt.bfloat16)
        nc.sync.dma_start(jpat[:], jpat_ext[:])
        ident_bf = const_pool.tile([128, 128], dt.bfloat16)
        make_identity(nc, ident_bf[:])
        ident_f32 = const_pool.tile([128, 128], dt.float32)
        make_identity(nc, ident_f32[:])

        skew_c = cpk[:, 0:16]     # (conservA+bconv) - transpose, flattened
        diss_c = cpk[:, 16:32]    # dissA + bdiss, flattened
        eye16 = cpk[:, 32:48]     # flattened I4
        readin_c = cpk[:, 48:52]
        writeout_c = cpk[:, 52:56]

        s_all = sm1_pool.tile([128, NT], dt.float32)
        proj_all = sm1_pool.tile([128, NT, NPROJ], dt.float32)
        E_all = sm1_pool.tile([128, NT, 16], dt.float32)
        E_grp = sm1_pool.tile([128, 4, NT, NS], dt.float32)
        c_all = sm1_pool.tile([128, NT, NS], dt.float32)
        ww_all = sm1_pool.tile([128, NT, NS], dt.float32)
        ss_all = sm1_pool.tile([128, NT, 4], dt.float32)

        def bcast(ap2d, shape):
            return ap2d.unsqueeze(1).broadcast_to(shape)

        x_bfs = {}

        x_grps = {}
        brTs = {}
        blks = {}

        p1_state = {}

        def p1_chunk(t, q):
            """load + cast + sum-of-squares for chunk q of tile t."""
            if q == 0:
                x_bf = xbb_pool.tile([128, IN_DIM], dt.bfloat16,
                                     tag="x_bf", name=f"x_bf{t}")
                x_bfs[t] = x_bf
            x_bf = x_bfs[t]
            ss = ss_all[:, t, :]
            xf = p1_pool.tile([128, EMB], dt.float32, tag="xf")
            eng = nc.sync if (t < TPB and q % 2 == 0) else nc.scalar
            eng.dma_start(
                xf[:], x_ext[t * 128:(t + 1) * 128,
                             q * EMB:(q + 1) * EMB])
            if q % 2 == 0:
                nc.vector.tensor_copy(x_bf[:, q * EMB:(q + 1) * EMB],
                                      xf[:])
                sqj = str_pool.tile([128, EMB], dt.bfloat16, tag="tmp")
                nc.scalar.activation(sqj[:], xf[:], Act.Square,
                                     accum_out=ss[:, q:q + 1])
            else:
                nc.scalar.activation(x_bf[:, q * EMB:(q + 1) * EMB],
                                     xf[:], Act.Copy)
                sqj = str_pool.tile([128, EMB], dt.bfloat16, tag="tmp")
                nc.vector.scalar_tensor_tensor(
                    out=sqj[:], in0=x_bf[:, q * EMB:(q + 1) * EMB],
                    scalar=1.0, in1=x_bf[:, q * EMB:(q + 1) * EMB],
                    op0=Alu.bypass, op1=Alu.mult,
                    accum_out=ss[:, q:q + 1])

        def p1_finish(t):
            ssum = small_pool.tile([128, 1], dt.float32, tag="s01")
            nc.vector.tensor_reduce(ssum[:], ss_all[:, t, :], Axis.X,
                                    Alu.add)
            nc.vector.tensor_scalar(
                out=ssum[:], in0=ssum[:], scalar1=1.0 / IN_DIM,
                scalar2=EPS, op0=Alu.mult, op1=Alu.add)
            sqr = small_pool.tile([128, 1], dt.float32, tag="sqr")
            nc.scalar.activation(sqr[:], ssum[:], Act.Sqrt)
            nc.vector.reciprocal(s_all[:, t:t + 1], sqr[:])

        def p1_tile(t):
            for q in range(4):
                p1_chunk(t, q)
            p1_finish(t)

        p3_state = {}

        def p3_start(g):
            p3_state[g] = ps_proj.tile([NPROJ, NB], dt.float32,
                                       tag="proj_ps", name=f"proj_ps{g}")

        def p3_part(g, c0, c1):
            """proj.T chunks [c0,c1) via on-chip PE transposes + matmul."""
            proj_ps = p3_state[g]
            for c2 in range(c0 // 2, c1 // 2):
                # transpose 2 chunks x 4 tiles into one bf16 psum bank
                tp = ps_br.tile([128, 8, 128], dt.bfloat16, tag="br_ps",
                                name=f"xtp{g}_{c2}")
                for cc in (2 * c2, 2 * c2 + 1):
                    for i in range(TPB):
                        nc.tensor.transpose(
                            tp[:, (cc % 2) * TPB + i, :],
                            x_bfs[g * TPB + i][:, cc * 128:(cc + 1) * 128],
                            ident_bf[:])
                xt = xt_pool.tile([128, 2, NB], dt.bfloat16, tag="xt")
                if c2 % 2 == 0:
                    nc.vector.tensor_copy(xt[:], tp[:])
                else:
                    nc.scalar.activation(xt[:], tp[:], Act.Copy)
                for cc in (2 * c2, 2 * c2 + 1):
                    nc.tensor.matmul(proj_ps[:], wcatT[:, cc, :],
                                     xt[:, cc % 2, :],
                                     start=(cc == 0), stop=(cc == NCH - 1))

        def p3_finish(g):
            proj_ps = p3_state.pop(g)
            projT = sm1_pool.tile([NPROJ, NB], dt.float32, tag="projT")
            nc.vector.tensor_copy(projT[:], proj_ps[:])
            for i in range(TPB):
                t = g * TPB + i
                tr_ps = ps_trp.tile([128, NPROJ], dt.float32, tag="tr_ps")
                nc.tensor.transpose(
                    tr_ps[:], projT[:, i * 128:(i + 1) * 128],
                    ident_f32[:NPROJ, :NPROJ])
                nc.vector.tensor_scalar(
                    out=proj_all[:, t, :], in0=tr_ps[:],
                    scalar1=s_all[:, t:t + 1], scalar2=None, op0=Alu.mult)

        def p3_proj(g):
            p3_start(g)
            p3_part(g, 0, NCH)
            p3_finish(g)

        def p4_smalls(g):
            """per-row generator math for block g, batched over TPB tiles."""
            pb = proj_all[:, g * TPB:(g + 1) * TPB, :]   # [128,TPB,42]

            def mm2(dst, lhs, rhs, rhs_pat):
                # batched per-row 4x4 matmul: loop j (broadcast dim) only
                pr = small_pool.tile([128, TPB, 64], dt.float32, tag="prod")
                pv5 = pr[:].rearrange("p t (i j k) -> p t i j k",
                                      j=NS, k=NS)
                lv = lhs[:].rearrange("p t (i k) -> p t i k", k=NS)
                rv = rhs[:].rearrange(rhs_pat, j=NS)
                for j in range(NS):
                    nc.vector.tensor_tensor(
                        pv5[:, :, :, j, :], lv,
                        rv[:, :, j, :].unsqueeze(2)
                        .broadcast_to([128, TPB, NS, NS]),
                        Alu.mult)
                nc.vector.tensor_reduce(
                    dst[:], pr[:].rearrange("p t (ij k) -> p t ij k", k=NS),
                    Axis.X, Alu.add)

            smw = small_pool.tile([128, TPB, 16], dt.float32, tag="smw")
            nc.vector.tensor_tensor(
                smw[:].rearrange("p t (i j) -> p t i j", j=NS),
                pb[:, :, 0:16].rearrange("p t (i j) -> p t i j", j=NS),
                pb[:, :, 0:16].rearrange("p t (j i) -> p t i j", i=NS),
                Alu.subtract)
            nc.vector.tensor_tensor(smw[:], smw[:],
                                    bcast(skew_c, [128, TPB, 16]), Alu.add)
            Rm = small_pool.tile([128, TPB, 16], dt.float32, tag="Rm")
            nc.vector.tensor_tensor(Rm[:], pb[:, :, 16:32],
                                    bcast(diss_c, [128, TPB, 16]), Alu.add)
            dtc = small_pool.tile([128, TPB, 1], dt.float32, tag="dtc")
            dtd = small_pool.tile([128, TPB, 1], dt.float32, tag="dtd")
            nc.scalar.activation(dtc[:], pb[:, :, 32:33], Act.Sigmoid,
                                 bias=cpk[:, 56:57])
            nc.scalar.activation(dtd[:], pb[:, :, 33:34], Act.Sigmoid,
                                 bias=cpk[:, 57:58])
            nc.vector.tensor_scalar(out=dtc[:], in0=dtc[:], scalar1=R_SIG,
                                    scalar2=C_SIG, op0=Alu.mult, op1=Alu.add)
            nc.vector.tensor_scalar(out=dtd[:], in0=dtd[:], scalar1=R_SIG,
                                    scalar2=C_SIG, op0=Alu.mult, op1=Alu.add)

            # K = R @ R^T
            Km = small_pool.tile([128, TPB, 16], dt.float32, tag="Km")
            mm2(Km, Rm, Rm, "p t (j k) -> p t j k")
            # A = dtc*skew - dtd*K  (batched: broadcast dt over the 16 cols)
            Am = small_pool.tile([128, TPB, 16], dt.float32, tag="Am")
            tmpA = small_pool.tile([128, TPB, 16], dt.float32, tag="tmpA")
            nc.vector.tensor_tensor(
                Am[:], smw[:], dtc[:].broadcast_to([128, TPB, 16]), Alu.mult)
            nc.vector.tensor_tensor(
                tmpA[:], Km[:], dtd[:].broadcast_to([128, TPB, 16]),
                Alu.mult)
            nc.vector.tensor_tensor(Am[:], Am[:], tmpA[:], Alu.subtract)
            # expm: order-4 Taylor (||A/16|| <= ~1.1 -> err < 2e-4) + 4 sq
            A2 = small_pool.tile([128, TPB, 16], dt.float32, tag="A2")
            A3 = small_pool.tile([128, TPB, 16], dt.float32, tag="A3")
            mm2(A2, Am, Am, "p t (k j) -> p t j k")
            mm2(A3, A2, Am, "p t (k j) -> p t j k")
            Em = small_pool.tile([128, TPB, 16], dt.float32, tag="Em")
            nc.vector.tensor_tensor(Em[:], Am[:],
                                    bcast(eye16, [128, TPB, 16]), Alu.add)
            nc.vector.scalar_tensor_tensor(
                out=Em[:], in0=A2[:], scalar=0.5, in1=Em[:],
                op0=Alu.mult, op1=Alu.add)
            nc.vector.scalar_tensor_tensor(
                out=Em[:], in0=A3[:], scalar=1.0 / 6.0, in1=Em[:],
                op0=Alu.mult, op1=Alu.add)
            E2 = small_pool.tile([128, TPB, 16], dt.float32, tag="E2")
            cur, nxt = Em, E2
            for q in range(4):
                if q == 3:
                    mm2(E_all[:, g * TPB:(g + 1) * TPB, :], cur, cur,
                        "p t (k j) -> p t j k")
                else:
                    mm2(nxt, cur, cur, "p t (k j) -> p t j k")
                    cur, nxt = nxt, cur
            Ev = E_all[:, g * TPB:(g + 1) * TPB, :]
            # rw / ww / c
            rw = small_pool.tile([128, TPB, NS], dt.float32, tag="rw")
            nc.vector.tensor_scalar(out=rw[:], in0=pb[:, :, 34:38],
                                    scalar1=scal["alpha_r"], scalar2=None,
                                    op0=Alu.mult)
            nc.vector.tensor_tensor(rw[:], rw[:],
                                    bcast(readin_c, [128, TPB, NS]), Alu.add)
            nc.scalar.activation(rw[:], rw[:], Act.Sigmoid)
            wws = ww_all[:, g * TPB:(g + 1) * TPB, :]
            nc.vector.tensor_scalar(out=wws, in0=pb[:, :, 38:42],
                                    scalar1=scal["alpha_w"], scalar2=None,
                                    op0=Alu.mult)
            nc.vector.tensor_tensor(wws, wws,
                                    bcast(writeout_c, [128, TPB, NS]),
                                    Alu.add)
            cprod = small_pool.tile([128, TPB, 16], dt.float32, tag="cprod")
            nc.vector.tensor_tensor(
                cprod[:].rearrange("p t (j n) -> p t j n", n=NS),
                Ev.rearrange("p t (n j) -> p t j n", j=NS),
                rw[:].unsqueeze(2).broadcast_to([128, TPB, NS, NS]),
                Alu.mult)
            nc.vector.tensor_reduce(
                c_all[:, g * TPB:(g + 1) * TPB, :],
                cprod[:].rearrange("p t (j n) -> p t j n", n=NS),
                Axis.X, Alu.add)
            # shuffle E into (j, b') partition layout for the P7 stationaries
            Ej = E_all[:].rearrange("p t (n j) -> p t n j", j=NS)
            t0 = g * TPB
            for j in range(NS):
                for gg in range(4):
                    nc.gpsimd.dma_start(
                        E_grp[j * 32:(j + 1) * 32, gg, t0:t0 + TPB, :],
                        Ej[gg * 32:(gg + 1) * 32, t0:t0 + TPB, :, j]
                        .unsqueeze(1))

        def p5_tile(t):
            x_bf = x_bfs.pop(t)
            # (j,b')-grouped copies of x for the P7 grouped matmuls,
            # issued early on the idle gpsimd queue to overlap P5/P6
            xgs = []
            for g in range(4):
                xg = xg_pool.tile([128, EMB], dt.bfloat16, tag="xg",
                                  name=f"xg{t}_{g}")
                xgs.append(xg)
                for j in range(NS):
                    nc.gpsimd.dma_start(
                        xg[j * 32:(j + 1) * 32, :],
                        x_bf[g * 32:(g + 1) * 32, j * EMB:(j + 1) * EMB])
            # ---- P5: branch = sum_j c_j x_j (TS mults + TT adds, 2x DVE) --
            br = str_pool.tile([128, EMB], dt.bfloat16, tag="br")
            nc.vector.tensor_scalar(
                out=br[:], in0=x_bf[:, 3 * EMB:4 * EMB],
                scalar1=c_all[:, t, 3:4], scalar2=None, op0=Alu.mult)
            for j in (2, 1, 0):
                tmp = str_pool.tile([128, EMB], dt.bfloat16, tag="tmp",
                                    name=f"tmp{t}_{j}")
                nc.scalar.activation(
                    tmp[:], x_bf[:, j * EMB:(j + 1) * EMB], Act.Identity,
                    scale=c_all[:, t, j:j + 1])
                nc.vector.tensor_tensor(br[:], br[:], tmp[:], Alu.add)
            # 16 PE transposes -> 2 batched psum banks -> fp8 brT
            brT = brt_pool.tile([128, 16, 128], dt.float8e4, tag="brT")
            for half in range(2):
                br_ps = ps_br.tile([128, 8, 128], dt.bfloat16, tag="br_ps")
                for h in range(8):
                    hh = half * 8 + h
                    nc.tensor.transpose(
                        br_ps[:, h, :], br[:, hh * 128:(hh + 1) * 128],
                        ident_bf[:])
                for quarter in range(2):
                    qq = half * 8 + quarter * 4
                    dst = brT[:, qq:qq + 4, :]
                    src = br_ps[:, quarter * 4:quarter * 4 + 4, :]
                    if quarter == 0:
                        nc.scalar.activation(dst, src, Act.Copy)
                    else:
                        nc.vector.tensor_copy(dst, src)
            x_grps[t] = xgs
            brTs[t] = brT
            # P7 stationary builds depend only on E_grp/ww_all (ready after
            # p4) -- run them here, off the P7 critical path
            jview = jpat[:].rearrange("p (n c) -> p n c", c=32)
            wwblk = blk_pool.tile([128, 128], dt.bfloat16, tag="wwblk",
                                  name=f"wwblk{t}")
            nc.vector.tensor_tensor(
                wwblk[:].rearrange("p (n c) -> p n c", c=32), jview,
                ww_all[:, t, :].unsqueeze(2).broadcast_to([128, NS, 32]),
                Alu.mult)
            eblks = []
            for g in range(4):
                eblk = blk_pool.tile([128, 128], dt.bfloat16, tag="eblk",
                                     name=f"eblk{t}_{g}")
                eblks.append(eblk)
                nc.vector.tensor_tensor(
                    eblk[:].rearrange("p (n c) -> p n c", c=32), jview,
                    E_grp[:, g, t, :].unsqueeze(2)
                    .broadcast_to([128, NS, 32]),
                    Alu.mult)
            blks[t] = (wwblk, eblks)

        def p67_tile(t):
            xgs = x_grps.pop(t)
            brT = brTs.pop(t)
            # ---- P6: y = branch @ W_mod.T via fp8 DoubleRow matmuls ----
            y_nb = str_pool.tile([128, EMB], dt.bfloat16, tag="y_nb")
            for eh in range(4):
                y_ps = ps_y.tile([128, 512], dt.float32, tag="y_ps")
                for c in range(8):
                    nc.tensor.matmul(
                        y_ps[:], brT[:, 2 * c:2 * c + 2, :],
                        wmodT[:, 2 * c:2 * c + 2,
                              eh * 512:(eh + 1) * 512],
                        start=(c == 0), stop=(c == 7), perf_mode=DR)
                if eh % 2 == 0:
                    nc.scalar.activation(y_nb[:, eh * 512:(eh + 1) * 512],
                                         y_ps[:], Act.Copy)
                else:
                    nc.vector.tensor_copy(y_nb[:, eh * 512:(eh + 1) * 512],
                                          y_ps[:])
            # ---- P7: out = E x + diag(ww) y via grouped PE matmuls ----
            # Per 32-row group g: stationary Eblk[(j,b'),(n,b'')] =
            # E_nj[32g+b'] * delta_{b'b''} contracts all 4 streams for 32
            # rows at full PE width; the ww_n y term accumulates into the
            # same psum via a K=32 matmul at partition offset 32g.
            wwblk, eblks = blks.pop(t)
            for g in range(4):
                gp = slice(g * 32, (g + 1) * 32)
                xg = xgs[g]
                ou = out_pool.tile([128, EMB], dt.float32, tag="ou")
                for k in range(4):
                    kc = slice(k * 512, (k + 1) * 512)
                    o_ps = ps_out.tile([128, 512], dt.float32, tag="y_ps")
                    nc.tensor.matmul(o_ps[:], eblks[g][:], xg[:, kc],
                                     start=True, stop=False)
                    nc.tensor.matmul(o_ps[:], wwblk[gp, :], y_nb[gp, kc],
                                     start=False, stop=True,
                                     tile_position=(g * 32, 0))
                    if (g + k) % 2 == 0:
                        nc.vector.tensor_copy(ou[:, kc], o_ps[:])
                    else:
                        nc.scalar.activation(ou[:, kc], o_ps[:], Act.Copy)
                r0 = t * 128 + g * 32
                for hf in range(2):
                    dsl = slice(hf * 1024, (hf + 1) * 1024)
                    nc.sync.dma_start(
                        out_ext[r0:r0 + 32, :, dsl].transpose([1, 0, 2]),
                        ou[:, dsl])

        # ---- schedule ----
        # Block 0: interleave per-chunk loads with P3 transposes+matmuls.
        p3_start(0)
        for q in range(4):
            for i in range(TPB):
                p1_chunk(i, q)
            p3_part(0, q * 16, (q + 1) * 16)
        for i in range(TPB):
            p1_finish(i)
        p3_finish(0)
        p4_smalls(0)
        if NBLK > 1:
            for i in range(TPB - 1):
                p5_tile(i)
                p67_tile(i)
                p1_tile(TPB + i)
            p5_tile(TPB - 1)
            p1_tile(2 * TPB - 1)
            # block 1's P3/P4 overlap tile TPB-1's P6/P7 on the PE
            p3_proj(1)
            p4_smalls(1)
            p67_tile(TPB - 1)
            for i in range(TPB):
                p5_tile(TPB + i)
                p67_tile(TPB + i)
        else:
            for i in range(TPB):
                p5_tile(i)
                p67_tile(i)

    nc.compile()
    return nc


def _prep_weights(inputs):
    W_conv = np.asarray(inputs["W_conv"], np.float32)
    W_diss = np.asarray(inputs["W_diss"], np.float32)
    W_dtc = np.asarray(inputs["W_dtc"], np.float32)
    W_dtd = np.asarray(inputs["W_dtd"], np.float32)
    W_read = np.asarray(inputs["W_read"], np.float32)
    W_write = np.asarray(inputs["W_write"], np.float32)
    W_mod = np.asarray(inputs["W_mod"], np.float32)

    Wcat = np.concatenate([W_conv, W_diss, W_dtc, W_dtd, W_read, W_write],
                          axis=0)
    assert Wcat.shape == (NPROJ, IN_DIM)
    wcatT = np.ascontiguousarray(
        Wcat.T.reshape(IN_DIM // 128, 128, NPROJ).transpose(1, 0, 2)
    ).astype(BF16)
    # [k-within-chunk, c, e]: element [p,c,e] = W_mod.T[c*128+p, e]
    wmodT = np.ascontiguousarray(
        W_mod.T.reshape(16, 128, EMB).transpose(1, 0, 2)
    ).astype(ml_dtypes.float8_e4m3)

    scal = dict(
        bias_c=float(np.asarray(inputs["log_dt_c"]).reshape(-1)[0]
                     + np.asarray(inputs["b_dtc"]).reshape(-1)[0]),
        bias_d=float(np.asarray(inputs["log_dt_d"]).reshape(-1)[0]
                     + np.asarray(inputs["b_dtd"]).reshape(-1)[0]),
        alpha_r=float(np.asarray(inputs["alpha_read_in"]).reshape(-1)[0]),
        alpha_w=float(np.asarray(inputs["alpha_write_out"]).reshape(-1)[0]),
    )

    cM = np.asarray(inputs["conserv_A"], np.float32) + \
        np.asarray(inputs["b_conv"], np.float32).reshape(NS, NS)
    skew_const = (cM - cM.T).reshape(-1)
    dissC = (np.asarray(inputs["diss_A"], np.float32) +
             np.asarray(inputs["b_diss"], np.float32).reshape(NS, NS)
             ).reshape(-1)
    eye16 = np.eye(NS, dtype=np.float32).reshape(-1)
    readin = np.asarray(inputs["read_in"], np.float32).reshape(-1)
    writeout = np.asarray(inputs["write_out"], np.float32).reshape(-1)
    cpack = np.concatenate([
        skew_const, dissC, eye16, readin, writeout,
        np.array([scal["bias_c"], scal["bias_d"]], np.float32)]
    ).astype(np.float32)
    assert cpack.shape == (58,)
    # block pattern for P7 stationaries: jpat[p, c] = (p % 32 == c % 32)
    pp = np.arange(128)
    jpat = (pp[:, None] % 32 == pp[None, :] % 32).astype(BF16)
    return wcatT, wmodT, cpack, jpat, scal


_NC_CACHE = {}


def kernel(**inputs):
    from concourse.bass_utils import run_bass_kernel_spmd

    x = np.asarray(inputs["x"], np.float32)
    B = x.shape[0]
    B_loc = B // N_CORES
    wcatT, wmodT, cpack, jpat, scal = _prep_weights(inputs)

    key = (B_loc, tuple(sorted(scal.items())))
    if key not in _NC_CACHE:
        _NC_CACHE[key] = _build(B_loc, scal)
    nc = _NC_CACHE[key]

    xf = x.reshape(B, IN_DIM)
    in_maps = []
    for i in range(N_CORES):
        in_maps.append({
            "x": np.ascontiguousarray(xf[i * B_loc:(i + 1) * B_loc]),
            "wcatT": wcatT,
            "wmodT": wmodT,
            "cpack": cpack,
            "jpat": jpat,
        })

    trace = os.environ.get("KERNEL_TRACE", "0") == "1"
    res = run_bass_kernel_spmd(nc, in_maps, core_ids=list(range(N_CORES)),
                               trace=trace)
    if trace and res.exec_time_ns is not None:
        print(f"HW exec time: {res.exec_time_ns} ns")
        kernel.last_exec_time_ns = res.exec_time_ns
    out = np.concatenate([res.results[i]["out"] for i in range(N_CORES)],
                         axis=0)
    return out
